# revision 2
# baseline (speedup 1.0000x reference)
"""DCellLinear v2: host bf16 pre-cast + HWDGE DMA-transpose loads.

y[s] = x[s] @ W[s].T + b[s] for 4096 subsystems, sharded 512/core over 8 cores.

vs v1 (SWDGE cast-load + PE transposes):
  - x/W/b are cast to bf16 on the HOST (numerically identical to v1, which
    cast f32->bf16 during the SWDGE load). Halves HBM read traffic:
    50.3 MB -> 25.2 MB per core.
  - x^T / W^T produced directly by HWDGE xbar DMA-transpose (bf16-only
    feature) -- eliminates all PE transposes, all PSUM->SBUF transpose
    copies, and all gpsimd/SWDGE use.
  - PE does only the real work: per subsystem-pair one [128]x[128,256]
    matmul in bf16 + rank-1 bias accumulate, exactly as v1.
  - Loads issue on the SP HWDGE ring (nc.sync), stores on the ACT ring
    (nc.scalar) so load and store descriptor streams don't serialize.
"""

import numpy as np
import ml_dtypes
from contextlib import ExitStack

import concourse.bass as bass
import concourse.mybir as mybir
from concourse.tile import TileContext
from concourse.bass_utils import run_bass_kernel_spmd

N_SUB, BATCH, D_IN, D_OUT = 4096, 64, 128, 128
N_CORES = 8
S_CORE = N_SUB // N_CORES          # 512 subsystems per core
CH = 64                            # subsystems per chunk
BF16 = mybir.dt.bfloat16
F32 = mybir.dt.float32


def build_nc(passes=1, ch=CH, sbuf_bufs=2, psum_bufs=6, split_waits=True,
             store_engine="scalar", extract_split=True, store_split=1):
    nchunk = S_CORE // ch
    xr = ch * BATCH                # x/y rows per chunk
    wr = ch * D_OUT                # W rows per chunk

    nc = bass.Bass()
    x_in = nc.declare_dram_parameter(
        "x", [S_CORE * BATCH, D_IN], BF16, isOutput=False)
    w_in = nc.declare_dram_parameter(
        "W", [S_CORE * D_OUT, D_IN], BF16, isOutput=False)
    b_in = nc.declare_dram_parameter(
        "b", [S_CORE, D_OUT], BF16, isOutput=False)
    ones_in = nc.declare_dram_parameter(
        "ones1", [1, 128], BF16, isOutput=False)
    y_out = nc.declare_dram_parameter(
        "out", [S_CORE * BATCH, D_OUT], F32, isOutput=True)

    st = getattr(nc, store_engine)

    with TileContext(nc) as tc, ExitStack() as ctx:
        consts = ctx.enter_context(tc.tile_pool(name="consts", bufs=1))
        xt_pool = ctx.enter_context(tc.tile_pool(name="xt_pool", bufs=sbuf_bufs))
        wt_pool = ctx.enter_context(tc.tile_pool(name="wt_pool", bufs=sbuf_bufs))
        bc_pool = ctx.enter_context(tc.tile_pool(name="bc_pool", bufs=sbuf_bufs))
        yc_pool = ctx.enter_context(tc.tile_pool(name="yc_pool", bufs=sbuf_bufs))
        py_pool = ctx.enter_context(tc.tile_pool(name="py_pool", bufs=psum_bufs, space="PSUM"))

        ones1 = consts.tile([1, 128], BF16)
        nc.sync.dma_start(out=ones1, in_=ones_in[:, :])

        # bc[0, s*128 + o] = b[c*ch + s, o]
        b_rows = b_in[:, :].rearrange("(c s) o -> c (s o)", s=ch)

        for c in [c for _ in range(passes) for c in range(nchunk)]:
            # xt[i, r] = x_row(c*xr + r)[i]  (xbar transpose during DMA)
            xt = xt_pool.tile([128, xr], BF16)
            nc.sync.dma_start(out=xt, in_=x_in[c * xr:(c + 1) * xr, :],
                              transpose=True)
            # wt[i, r] = W_row(c*wr + r)[i]
            wt = wt_pool.tile([128, wr], BF16)
            nc.sync.dma_start(out=wt, in_=w_in[c * wr:(c + 1) * wr, :],
                              transpose=True)
            bc = bc_pool.tile([1, ch * 128], BF16)
            nc.sync.dma_start(out=bc, in_=b_rows[c:c + 1, :])

            # yc[p, g, o] = y row (c*xr + 128g + p), col o
            yc = yc_pool.tile([128, ch // 2, 128], F32)
            nbank = ch // 4               # 2 pairs (4 subsystems) per PSUM bank
            binc = nbank // store_split   # banks per store piece
            for h in range(nbank):
                yp = py_pool.tile([128, 2, 2, 128], F32)  # 1 bank
                for j in range(2):
                    g = 2 * h + j         # pair index within chunk
                    # stationary: x rows 128g..128g+127 (subsys 2g|2g+1 batches)
                    lhs = xt[:, 128 * g:128 * g + 128]
                    # moving: W rows 256g..256g+255 (subsys 2g then 2g+1)
                    rhs = wt[:, 256 * g:256 * g + 256]
                    nc.tensor.matmul(yp[:, j, :, :], lhs, rhs,
                                     start=(j == 0), stop=False)
                # rank-1 bias for the 4 subsystems in this bank
                nc.tensor.matmul(yp[:, :, :, :], ones1,
                                 bc[0:1, h * 512:(h + 1) * 512],
                                 start=False, stop=True)
                # diagonal extraction (useful halves only)
                if extract_split:
                    nc.vector.tensor_copy(yc[0:64, 2 * h:2 * h + 2, :],
                                          yp[0:64, :, 0, :])
                else:
                    nc.scalar.copy(yc[0:64, 2 * h:2 * h + 2, :],
                                   yp[0:64, :, 0, :])
                nc.scalar.copy(yc[64:128, 2 * h:2 * h + 2, :],
                               yp[64:128, :, 1, :])
                if (h + 1) % binc == 0:
                    # store the finished slice: y rows with g in [gs, ge)
                    gs, ge = 2 * (h + 1 - binc), 2 * (h + 1)
                    y_dst = y_out[c * xr + 128 * gs:c * xr + 128 * ge, :]
                    st.dma_start(
                        out=y_dst.rearrange("(g p) o -> p g o", p=128),
                        in_=yc[:, gs:ge, :])

    if split_waits:
        _split_excess_waits(nc)
    return nc


# Walrus codegen allows only one sync-wait slot on engine-compute
# instructions; Tile's scheduler can emit several. Hoist extras onto
# same-engine NoOps inserted just before the instruction.
_WAIT_EXEMPT = {
    "InstCall", "InstUnconditionalBranch",
    "InstEventSemaphore", "InstISA", "InstHalt",
}


def _split_excess_waits(nc, max_waits=1):
    import concourse.mybir as mybir_
    k = 0
    for f in nc.m.functions:
        for blk in f.blocks:
            out = []
            changed = False
            for inst in blk.instructions:
                si = getattr(inst, "sync_info", None)
                if (si is not None and si.on_wait and len(si.on_wait) > max_waits
                        and type(inst).__name__ not in _WAIT_EXEMPT):
                    waits = list(si.on_wait)
                    for w in waits[:-max_waits]:
                        nop = mybir_.InstNoOp(name=f"I-nopw{k}")
                        k += 1
                        nop.engine = inst.engine
                        nop.sync_info = mybir_.SyncInfo(on_wait=[w], on_update=[])
                        out.append(nop)
                    inst.sync_info = mybir_.SyncInfo(
                        on_wait=waits[-max_waits:], on_update=list(si.on_update))
                    changed = True
                out.append(inst)
            if changed:
                blk.instructions = out
    return nc


_CACHE = {}


def _get_nc():
    if "nc" not in _CACHE:
        _CACHE["nc"] = build_nc()
    return _CACHE["nc"]


def _in_maps(x, W, b):
    bf = ml_dtypes.bfloat16
    ones1 = np.ones((1, 128), dtype=bf)
    maps = []
    for i in range(N_CORES):
        sl = slice(i * S_CORE, (i + 1) * S_CORE)
        maps.append({
            "x": np.asarray(x[sl]).reshape(S_CORE * BATCH, D_IN).astype(bf),
            "W": np.asarray(W[sl]).reshape(S_CORE * D_OUT, D_IN).astype(bf),
            "b": np.asarray(b[sl]).astype(bf),
            "ones1": ones1,
        })
    return maps


def _run(x, W, b, trace=False, **kw):
    x = np.asarray(x, dtype=np.float32)
    W = np.asarray(W, dtype=np.float32)
    b = np.asarray(b, dtype=np.float32)
    res = run_bass_kernel_spmd(
        _get_nc(), _in_maps(x, W, b), core_ids=list(range(N_CORES)),
        trace=trace, **kw)
    y = np.concatenate([res.results[i]["out"] for i in range(N_CORES)], axis=0)
    return y.astype(np.float32, copy=False), res


def kernel(x, W, b):
    y, _ = _run(x, W, b, trace=False)
    return y
